# revision 26
# baseline (speedup 1.0000x reference)
"""Trainium2 Bass kernel for CustomRNN — 8-core fp8-DoubleRow + bf16-tail, v2.

Reference computation (all fp32):
    xproj = einsum('bti,ih->bth', x, Wxh) + b_xh          # B,T,HID
    h_{t+1} = tanh(xproj[:,t] + h_t @ Whh + b_hh)         # scan over T
    out = h_T @ Why + b_y                                  # B,NC

Strategy:
  - Batch-parallel over 8 NeuronCores, BS=128 per core; layout-swapped
    matmuls (stationary x_t^T / h_t^T, moving weight rows, N=512).
  - The tanh recurrence is contractive (per-step error gain ~0.5):
    fp8(e4m3) operands for the first T-8 steps + bf16 for the last 8
    gives absmax-rel ~5e-3 (measured in sim), same as all-bf16.
  - fp8 matmuls use DoubleRow perf mode ([128, 2, free] k-subtile
    pairs, 2 contraction rows/cycle).  PSUM accumulation stays fp32.
  - v3 pipeline: the per-step serial loop (recurrence -> evacuate ->
    transpose -> tanh -> next recurrence) is pipelined at PAIR
    granularity: the evacuation runs as 4 [128,256] chunks split
    across DVE (half0) and GpSimd (half1) in parallel, each feeding
    its 2 PE transposes and one [128,256] tanh as soon as it lands.
    The recurrence order kp0n0 kp0n1 kp1n0 kp1n1 kp2n0 kp3n0 kp2n1
    kp3n1 completes PSUM half0 two matmuls early, so chunk 0's
    evac/transpose/tanh overlaps the recurrence tail and the next
    step's recurrence starts ~1.5us after the previous one ends
    instead of ~2.1us.  xproj(t+1) DoubleRow matmuls sit between the
    transpose pairs in the PE queue as bubble filler.
"""

import os
import numpy as np

B, T, IN, HID, NCLS = 1024, 128, 512, 1024, 10
P = 128
NCORES = int(os.environ.get("RNN_NCORES", "8"))
BS = B // NCORES                 # per-core batch (stationary M dim)
JT = HID // P                    # hidden tiles
KT = HID // P                    # contraction tiles over h
IT = IN // P                     # contraction tiles over x
NH = 512                         # moving-dim chunk (PSUM bank, fp32)
HN = HID // NH                   # number of N-chunks (2)
KH = KT // HN                    # k-tiles per half (4)
T_STEPS = int(os.environ.get("RNN_T", str(T)))
KTAIL = int(os.environ.get("RNN_KTAIL", "6"))    # bf16 tail steps
T8 = max(0, T_STEPS - KTAIL)     # fp8 steps

_CACHE = {}


def _build_nc():
    from concourse import bacc, mybir

    f32 = mybir.dt.float32
    bf16 = mybir.dt.bfloat16
    fp8 = mybir.dt.float8e4
    nc = bacc.Bacc(
        "TRN2", target_bir_lowering=False, debug=False, num_devices=NCORES
    )

    xT8 = (
        nc.declare_dram_parameter("xT8", [T8, IN, BS], fp8, isOutput=False)
        if T8 > 0
        else None
    )
    xT16 = (
        nc.declare_dram_parameter(
            "xT16", [T_STEPS - T8, IN, BS], bf16, isOutput=False
        )
        if T_STEPS > T8
        else None
    )
    wxh = nc.declare_dram_parameter("wxh", [IN, HID], bf16, isOutput=False)
    whh = nc.declare_dram_parameter("whh", [HID, HID], bf16, isOutput=False)
    wxh8 = nc.declare_dram_parameter("wxh8", [IN, HID], fp8, isOutput=False)
    whh8 = nc.declare_dram_parameter("whh8", [HID, HID], fp8, isOutput=False)
    biasrep = nc.declare_dram_parameter("biasrep", [P, HID], f32, isOutput=False)
    why = nc.declare_dram_parameter("why", [HID, NCLS], bf16, isOutput=False)
    b_y = nc.declare_dram_parameter("b_y", [1, NCLS], bf16, isOutput=False)
    ones = nc.declare_dram_parameter("ones", [1, P], bf16, isOutput=False)
    ident = nc.declare_dram_parameter("ident", [P, P], bf16, isOutput=False)
    out = nc.declare_dram_parameter("out", [BS, NCLS], f32, isOutput=True)
    return nc, (xT8, xT16, wxh, whh, wxh8, whh8, biasrep, why, b_y, ones, ident, out)


def _emit(nc, tensors):
    from contextlib import ExitStack

    import concourse.bass as bass
    import concourse.tile as tile
    from concourse import mybir

    f32 = mybir.dt.float32
    bf16 = mybir.dt.bfloat16
    fp8 = mybir.dt.float8e4
    DR = mybir.MatmulPerfMode.DoubleRow
    TANH = mybir.ActivationFunctionType.Tanh
    ts = bass.ts
    (xT8, xT16, wxh, whh, wxh8, whh8, biasrep, why, b_y, ones, ident, out) = tensors

    with tile.TileContext(nc) as tc, ExitStack() as ctx:
        const = ctx.enter_context(tc.tile_pool(name="const", bufs=1))
        xpool = ctx.enter_context(tc.tile_pool(name="x", bufs=8))
        hpool = ctx.enter_context(tc.tile_pool(name="h", bufs=2))
        pspool = ctx.enter_context(tc.tile_pool(name="ps", bufs=4, space="PSUM"))
        psTpool = ctx.enter_context(tc.tile_pool(name="psT", bufs=4, space="PSUM"))
        opool = ctx.enter_context(tc.tile_pool(name="o", bufs=2))

        # --- persistent SBUF ---
        whh_sb = [const.tile([P, HID], bf16, name=f"whh{k}", tag=f"whh{k}") for k in range(KT)]
        wxh_sb = [const.tile([P, HID], bf16, name=f"wxh{i}", tag=f"wxh{i}") for i in range(IT)]
        whh8_sb = const.tile([P, KT, HID], fp8, name="whh8_sb", tag="whh8")
        wxh8_sb = const.tile([P, IT, HID], fp8, name="wxh8_sb", tag="wxh8")
        biasrep_sb = const.tile([P, HID], f32, name="biasrep_sb", tag="biasrep")
        why_sb = [const.tile([P, NCLS], bf16, name=f"why{k}", tag=f"why{k}") for k in range(KT)]
        by_sb = const.tile([1, NCLS], bf16, name="by_sb", tag="by")
        ones_sb = const.tile([1, P], bf16, name="ones_sb", tag="ones")
        ident_sb = const.tile([P, P], bf16, name="ident_sb", tag="ident")
        # h_t^T stationary tiles, [P, KH, BS] per N-half so adjacent
        # k-tiles are contiguous (DoubleRow consumes [P, 2, BS] pairs).
        hT8_sb = [
            [const.tile([P, KH, BS], fp8, name=f"hT8_{p}_{n}", tag=f"hT8_{p}_{n}") for n in range(HN)]
            for p in range(2)
        ]
        hT16_sb = [
            [const.tile([P, KH, BS], bf16, name=f"hT16_{p}_{n}", tag=f"hT16_{p}_{n}") for n in range(HN)]
            for p in range(2)
        ]

        def hT16_slice(p, k):
            return hT16_sb[p][k // KH][:, k % KH, :]

        # Startup DMAs.  fp8 weights first (needed at t=0/t=1); bf16
        # copies are not needed until step T8, stream them last.
        for i in range(IT):
            nc.scalar.dma_start(wxh8_sb[:, i, :], wxh8[ts(i, P), :])
        nc.scalar.dma_start(biasrep_sb[:], biasrep[:, :])
        for k in range(KT):
            eng = nc.gpsimd if k % 2 == 0 else nc.scalar
            eng.dma_start(whh8_sb[:, k, :], whh8[ts(k, P), :])
        nc.gpsimd.dma_start(by_sb[:], b_y[:, :])
        nc.gpsimd.dma_start(ones_sb[:], ones[:, :])
        nc.gpsimd.dma_start(ident_sb[:], ident[:, :])
        for k in range(KT):
            nc.gpsimd.dma_start(why_sb[k][:], why[ts(k, P), :])
        # background: bf16 tail weights (not needed until step T8)
        for i in range(IT):
            nc.gpsimd.dma_start(wxh_sb[i][:], wxh[ts(i, P), :])
        for k in range(KT):
            nc.gpsimd.dma_start(whh_sb[k][:], whh[ts(k, P), :])

        def dma_x(t):
            """Allocate + start the x^T DMA for step t; returns the tile."""
            if t < T8:
                xt = xpool.tile([P, IT, BS], fp8, name="xt8", tag="xt8")
                nc.sync.dma_start(
                    xt[:], xT8[t, :, :].rearrange("(i p) b -> p i b", p=P)
                )
            else:
                xt = xpool.tile([P, IT, BS], bf16, name="xt16", tag="xt16")
                nc.sync.dma_start(
                    xt[:], xT16[t - T8, :, :].rearrange("(i p) b -> p i b", p=P)
                )
            return xt

        def emit_xproj(t, xt, ps, part=None):
            """Input-projection matmuls for step t into ps (start=True).
            part=0/1 emits only the first/second half of the contraction."""
            if t < T8:
                ips = range(IT // 2) if part is None else [part]
                for ip in ips:
                    for n in range(HN):
                        nc.tensor.matmul(
                            ps[n][:],
                            xt[:, 2 * ip : 2 * ip + 2, :],
                            wxh8_sb[:, 2 * ip : 2 * ip + 2, ts(n, NH)],
                            start=(ip == 0),
                            stop=(t == 0 and ip == IT // 2 - 1),
                            perf_mode=DR,
                        )
            else:
                iis = range(IT) if part is None else range(2 * part, 2 * part + 2)
                for i in iis:
                    for n in range(HN):
                        nc.tensor.matmul(
                            ps[n][:],
                            xt[:, i, :],
                            wxh_sb[i][:, ts(n, NH)],
                            start=(i == 0),
                            stop=(t == 0 and i == IT - 1),
                        )

        # --- software-pipelined recurrence ---
        # Iteration t PE order: [T(t) half0] [A(t+1)] [T(t) half1] [C(t)].
        xt_next = dma_x(0)
        if T_STEPS > 1:
            xt_next2 = dma_x(1)
        ps_next = [pspool.tile([P, NH], f32, name=f"ps{n}", tag="ps") for n in range(HN)]
        emit_xproj(0, xt_next, ps_next)

        prev_pre = None
        for t in range(T_STEPS):
            f8 = t < T8
            ps = ps_next
            # prefetch x for t+2 (2-step DMA lead)
            if t + 2 < T_STEPS:
                xt_next, xt_next2 = xt_next2, dma_x(t + 2)
            elif t + 1 < T_STEPS:
                xt_next = xt_next2

            if t > 0:
                hT_dst = hT8_sb if f8 else hT16_sb
                psT = [
                    psTpool.tile([P, KH * P], bf16, name=f"psT{n}", tag="psT")
                    for n in range(HN)
                ]
                # transposes, half 0
                for j in range(KH):
                    nc.tensor.transpose(
                        psT[0][:, ts(j, P)], prev_pre[:, ts(j, P)], ident_sb[:]
                    )
            # A(t+1): next step's input projection (independent PE work
            # that covers the evac->transpose->tanh chain of step t)
            if t + 1 < T_STEPS:
                ps_next = [
                    pspool.tile([P, NH], f32, name=f"ps{n}", tag="ps")
                    for n in range(HN)
                ]
                emit_xproj(t + 1, xt_next, ps_next)
            if t > 0:
                # transposes, half 1
                for j in range(KH):
                    nc.tensor.transpose(
                        psT[1][:, ts(j, P)],
                        prev_pre[:, ts(KH + j, P)],
                        ident_sb[:],
                    )
                # tanh: one [128,512] ACTIVATE per half — fewer PSUM-init
                # charges, tanh chain ends ~600ns sooner than 4 per-pair
                # ACTs; first-pair delivery stays ahead of the recurrence
                # start (which is PE-queue-bound).
                for half in range(HN):
                    nc.scalar.activation(
                        hT_dst[t % 2][half][:],
                        psT[half][:].rearrange("p (k b) -> p k b", k=KH),
                        TANH,
                    )
                # C(t): recurrence.  Pair-major, n-interleaved, with n0
                # finishing two matmuls early for evac overlap.
                if f8:
                    order = [(0, 0), (0, 1), (1, 0), (1, 1), (2, 0), (3, 0), (2, 1), (3, 1)]
                    last = {0: (3, 0), 1: (3, 1)}
                    for kp, n in order:
                        buf = hT8_sb[t % 2][kp // 2]
                        nc.tensor.matmul(
                            ps[n][:],
                            buf[:, 2 * (kp % 2) : 2 * (kp % 2) + 2, :],
                            whh8_sb[:, 2 * kp : 2 * kp + 2, ts(n, NH)],
                            start=False,
                            stop=((kp, n) == last[n]),
                            perf_mode=DR,
                        )
                else:
                    order = [(k, n) for k in range(6) for n in range(HN)]
                    order += [(6, 0), (7, 0), (6, 1), (7, 1)]
                    last = {0: (7, 0), 1: (7, 1)}
                    for k, n in order:
                        nc.tensor.matmul(
                            ps[n][:],
                            hT16_slice(t % 2, k),
                            whh_sb[k][:, ts(n, NH)],
                            start=False,
                            stop=((k, n) == last[n]),
                        )
            # D(t): evacuate+bias on DVE per half (half0 first — its bank
            # completes two matmuls early and gates the next transposes)
            pre = hpool.tile([P, HID], bf16, name="pre", tag="pre")
            for n in range(HN):
                nc.vector.tensor_add(
                    pre[:, ts(n, NH)], ps[n][:], biasrep_sb[:, ts(n, NH)]
                )
            prev_pre = pre

        # --- final tanh + logits: out[b, n] = h_T[b, :] @ Why + b_y ---
        pb = T_STEPS % 2
        psTf = [
            psTpool.tile([P, KH * P], bf16, name=f"psTf{n}", tag="psT")
            for n in range(HN)
        ]
        for j in range(JT):
            nc.tensor.transpose(
                psTf[j // KH][:, ts(j % KH, P)], prev_pre[:, ts(j, P)], ident_sb[:]
            )
        for pp in range(JT // 2):
            half = pp // 2
            nc.scalar.activation(
                hT16_sb[pb][half][:, 2 * (pp % 2) : 2 * (pp % 2) + 2, :],
                psTf[half][:, ts(pp % 2, 2 * P)].rearrange("p (k b) -> p k b", k=2),
                TANH,
            )
        ps_l = pspool.tile([P, NCLS], f32, name="ps_l", tag="ps")
        for k in range(KT):
            nc.tensor.matmul(
                ps_l[:],
                hT16_slice(pb, k),
                why_sb[k][:],
                start=(k == 0),
                stop=False,
            )
        # broadcast-add b_y via a K=1 matmul: ones[1,P].T @ b_y[1,N]
        nc.tensor.matmul(ps_l[:], ones_sb[:], by_sb[:], start=False, stop=True)
        osb = opool.tile([P, NCLS], f32, name="osb", tag="osb")
        nc.vector.tensor_copy(osb[:], ps_l[:])
        nc.sync.dma_start(out[:, :], osb[:])


def _get_program():
    if "nc" not in _CACHE:
        nc, tensors = _build_nc()
        _emit(nc, tensors)
        nc.compile()
        _CACHE["nc"] = nc
    return _CACHE["nc"]


def _make_in_maps(x, Wxh, b_xh, Whh, b_hh, Why, b_y):
    import ml_dtypes

    bf = ml_dtypes.bfloat16
    f8 = ml_dtypes.float8_e4m3
    x = np.asarray(x, dtype=np.float32)
    Wxh_q = np.asarray(Wxh, np.float32).astype(bf)
    Whh_q = np.asarray(Whh, np.float32).astype(bf)
    Wxh8_q = np.asarray(Wxh, np.float32).astype(f8)
    Whh8_q = np.asarray(Whh, np.float32).astype(f8)
    Why_q = np.asarray(Why, np.float32).astype(bf)
    b_y_q = np.asarray(b_y, np.float32).reshape(1, NCLS).astype(bf)
    bias = np.asarray(b_xh, np.float32) + np.asarray(b_hh, np.float32)
    biasrep = np.ascontiguousarray(
        np.broadcast_to(bias.reshape(1, HID), (P, HID)), dtype=np.float32
    )
    ones_row = np.ones((1, P), dtype=bf)
    ident_np = np.eye(P, dtype=np.float32).astype(bf)

    # host-side layout marshaling: [B,T,IN] -> [T,IN,B], shard batch
    xT_full = np.transpose(x[:, :T_STEPS, :], (1, 2, 0))
    xT8_full = xT_full[:T8].astype(f8)
    xT16_full = xT_full[T8:].astype(bf)

    in_maps = []
    for c in range(NCORES):
        m = {
            "wxh": Wxh_q,
            "whh": Whh_q,
            "wxh8": Wxh8_q,
            "whh8": Whh8_q,
            "biasrep": biasrep,
            "why": Why_q,
            "b_y": b_y_q,
            "ones": ones_row,
            "ident": ident_np,
        }
        if T8 > 0:
            m["xT8"] = np.ascontiguousarray(xT8_full[:, :, c * BS : (c + 1) * BS])
        if T_STEPS > T8:
            m["xT16"] = np.ascontiguousarray(xT16_full[:, :, c * BS : (c + 1) * BS])
        in_maps.append(m)
    return in_maps


def kernel(x, Wxh, b_xh, Whh, b_hh, Why, b_y):
    from concourse.bass_utils import run_bass_kernel_spmd

    in_maps = _make_in_maps(x, Wxh, b_xh, Whh, b_hh, Why, b_y)
    nc = _get_program()
    res = run_bass_kernel_spmd(nc, in_maps, list(range(NCORES)))
    _CACHE["last_results"] = res
    return np.concatenate([res.results[c]["out"] for c in range(NCORES)], axis=0)


def bench_hw(inputs, iters=8):
    """Steady-state per-execution wall time with device-resident inputs."""
    import time as _time

    import jax
    from jax.sharding import Mesh, NamedSharding, PartitionSpec
    from jax.experimental.shard_map import shard_map
    from concourse import bass2jax, mybir

    in_maps = _make_in_maps(**inputs)
    nc = _get_program()
    bass2jax.install_neuronx_cc_hook()

    partition_name = (
        nc.partition_id_tensor.name if nc.partition_id_tensor else None
    )
    in_names, out_names, out_avals, zero_outs = [], [], [], []
    for alloc in nc.m.functions[0].allocations:
        if not isinstance(alloc, mybir.MemoryLocationSet):
            continue
        name = alloc.memorylocations[0].name
        if alloc.kind == "ExternalInput":
            if name != partition_name:
                in_names.append(name)
        elif alloc.kind == "ExternalOutput":
            out_names.append(name)
            np_dt = mybir.dt.np(alloc.dtype)
            out_avals.append(
                jax.core.ShapedArray(tuple(alloc.tensor_shape), np_dt)
            )
            zero_outs.append(np.zeros(tuple(alloc.tensor_shape), np_dt))
    n_params = len(in_names)
    all_in_names = in_names + out_names
    if partition_name is not None:
        all_in_names = all_in_names + [partition_name]

    def _body(*args):
        operands = list(args)
        if partition_name is not None:
            operands.append(bass2jax.partition_id_tensor())
        outs = bass2jax._bass_exec_p.bind(
            *operands,
            out_avals=tuple(out_avals),
            in_names=tuple(all_in_names),
            out_names=tuple(out_names),
            lowering_input_output_aliases=(),
            sim_require_finite=True,
            sim_require_nnan=True,
            nc=nc,
        )
        return tuple(outs)

    devices = jax.devices()[:NCORES]
    mesh = Mesh(np.asarray(devices), ("core",))
    spec = PartitionSpec("core")
    sharded = jax.jit(
        shard_map(
            _body,
            mesh=mesh,
            in_specs=(spec,) * (n_params + len(out_names)),
            out_specs=(spec,) * len(out_names),
            check_rep=False,
        ),
        keep_unused=True,
    )
    concat_in = [
        jax.device_put(
            np.concatenate([np.asarray(m[nm]) for m in in_maps], axis=0),
            NamedSharding(mesh, spec),
        )
        for nm in in_names
    ]
    concat_zero = [
        jax.device_put(
            np.zeros((NCORES * z.shape[0], *z.shape[1:]), z.dtype),
            NamedSharding(mesh, spec),
        )
        for z in zero_outs
    ]
    jax.block_until_ready(concat_in)

    # warmup (compile)
    outs = sharded(*concat_in, *concat_zero)
    jax.block_until_ready(outs)

    times = []
    for _ in range(iters):
        t0 = _time.perf_counter()
        outs = sharded(*concat_in, *concat_zero)
        jax.block_until_ready(outs)
        times.append((_time.perf_counter() - t0) * 1e9)
    result = np.concatenate(
        [
            np.asarray(outs[0]).reshape(NCORES, BS, NCLS)[c]
            for c in range(NCORES)
        ],
        axis=0,
    )
    return min(times), times, result


# revision 28
# speedup vs baseline: 1.0126x; 1.0126x over previous
"""Trainium2 Bass kernel for CustomRNN — 8-core fp8-DoubleRow + bf16-tail, v2.

Reference computation (all fp32):
    xproj = einsum('bti,ih->bth', x, Wxh) + b_xh          # B,T,HID
    h_{t+1} = tanh(xproj[:,t] + h_t @ Whh + b_hh)         # scan over T
    out = h_T @ Why + b_y                                  # B,NC

Strategy:
  - Batch-parallel over 8 NeuronCores, BS=128 per core; layout-swapped
    matmuls (stationary x_t^T / h_t^T, moving weight rows, N=512).
  - The tanh recurrence is contractive (per-step error gain ~0.5):
    fp8(e4m3) operands for the first T-8 steps + bf16 for the last 8
    gives absmax-rel ~5e-3 (measured in sim), same as all-bf16.
  - fp8 matmuls use DoubleRow perf mode ([128, 2, free] k-subtile
    pairs, 2 contraction rows/cycle).  PSUM accumulation stays fp32.
  - v3 pipeline: the per-step serial loop (recurrence -> evacuate ->
    transpose -> tanh -> next recurrence) is pipelined at PAIR
    granularity: the evacuation runs as 4 [128,256] chunks split
    across DVE (half0) and GpSimd (half1) in parallel, each feeding
    its 2 PE transposes and one [128,256] tanh as soon as it lands.
    The recurrence order kp0n0 kp0n1 kp1n0 kp1n1 kp2n0 kp3n0 kp2n1
    kp3n1 completes PSUM half0 two matmuls early, so chunk 0's
    evac/transpose/tanh overlaps the recurrence tail and the next
    step's recurrence starts ~1.5us after the previous one ends
    instead of ~2.1us.  xproj(t+1) DoubleRow matmuls sit between the
    transpose pairs in the PE queue as bubble filler.
"""

import os
import numpy as np

B, T, IN, HID, NCLS = 1024, 128, 512, 1024, 10
P = 128
NCORES = int(os.environ.get("RNN_NCORES", "8"))
BS = B // NCORES                 # per-core batch (stationary M dim)
JT = HID // P                    # hidden tiles
KT = HID // P                    # contraction tiles over h
IT = IN // P                     # contraction tiles over x
NH = 512                         # moving-dim chunk (PSUM bank, fp32)
HN = HID // NH                   # number of N-chunks (2)
KH = KT // HN                    # k-tiles per half (4)
T_STEPS = int(os.environ.get("RNN_T", str(T)))
KTAIL = int(os.environ.get("RNN_KTAIL", "6"))    # bf16 tail steps
T8 = max(0, T_STEPS - KTAIL)     # fp8 steps

_CACHE = {}


def _build_nc():
    from concourse import bacc, mybir

    f32 = mybir.dt.float32
    bf16 = mybir.dt.bfloat16
    fp8 = mybir.dt.float8e4
    nc = bacc.Bacc(
        "TRN2", target_bir_lowering=False, debug=False, num_devices=NCORES
    )

    xT8 = (
        nc.declare_dram_parameter("xT8", [T8, IN, BS], fp8, isOutput=False)
        if T8 > 0
        else None
    )
    xT16 = (
        nc.declare_dram_parameter(
            "xT16", [T_STEPS - T8, IN, BS], bf16, isOutput=False
        )
        if T_STEPS > T8
        else None
    )
    wxh = nc.declare_dram_parameter("wxh", [IN, HID], bf16, isOutput=False)
    whh = nc.declare_dram_parameter("whh", [HID, HID], bf16, isOutput=False)
    wxh8 = nc.declare_dram_parameter("wxh8", [IN, HID], fp8, isOutput=False)
    whh8 = nc.declare_dram_parameter("whh8", [HID, HID], fp8, isOutput=False)
    biasrep = nc.declare_dram_parameter("biasrep", [P, HID], f32, isOutput=False)
    why = nc.declare_dram_parameter("why", [HID, NCLS], bf16, isOutput=False)
    b_y = nc.declare_dram_parameter("b_y", [1, NCLS], bf16, isOutput=False)
    ones = nc.declare_dram_parameter("ones", [1, P], bf16, isOutput=False)
    ident = nc.declare_dram_parameter("ident", [P, P], bf16, isOutput=False)
    out = nc.declare_dram_parameter("out", [BS, NCLS], f32, isOutput=True)
    return nc, (xT8, xT16, wxh, whh, wxh8, whh8, biasrep, why, b_y, ones, ident, out)


def _emit(nc, tensors):
    from contextlib import ExitStack

    import concourse.bass as bass
    import concourse.tile as tile
    from concourse import mybir

    f32 = mybir.dt.float32
    bf16 = mybir.dt.bfloat16
    fp8 = mybir.dt.float8e4
    DR = mybir.MatmulPerfMode.DoubleRow
    TANH = mybir.ActivationFunctionType.Tanh
    ts = bass.ts
    (xT8, xT16, wxh, whh, wxh8, whh8, biasrep, why, b_y, ones, ident, out) = tensors

    with tile.TileContext(nc) as tc, ExitStack() as ctx:
        const = ctx.enter_context(tc.tile_pool(name="const", bufs=1))
        xpool = ctx.enter_context(tc.tile_pool(name="x", bufs=8))
        hpool = ctx.enter_context(tc.tile_pool(name="h", bufs=2))
        pspool = ctx.enter_context(tc.tile_pool(name="ps", bufs=4, space="PSUM"))
        psTpool = ctx.enter_context(tc.tile_pool(name="psT", bufs=4, space="PSUM"))
        opool = ctx.enter_context(tc.tile_pool(name="o", bufs=2))

        # --- persistent SBUF ---
        whh_sb = [const.tile([P, HID], bf16, name=f"whh{k}", tag=f"whh{k}") for k in range(KT)]
        wxh_sb = [const.tile([P, HID], bf16, name=f"wxh{i}", tag=f"wxh{i}") for i in range(IT)]
        whh8_sb = const.tile([P, KT, HID], fp8, name="whh8_sb", tag="whh8")
        wxh8_sb = const.tile([P, IT, HID], fp8, name="wxh8_sb", tag="wxh8")
        biasrep_sb = const.tile([P, HID], f32, name="biasrep_sb", tag="biasrep")
        why_sb = [const.tile([P, NCLS], bf16, name=f"why{k}", tag=f"why{k}") for k in range(KT)]
        by_sb = const.tile([1, NCLS], bf16, name="by_sb", tag="by")
        ones_sb = const.tile([1, P], bf16, name="ones_sb", tag="ones")
        ident_sb = const.tile([P, P], bf16, name="ident_sb", tag="ident")
        # h_t^T stationary tiles, [P, KH, BS] per N-half so adjacent
        # k-tiles are contiguous (DoubleRow consumes [P, 2, BS] pairs).
        hT8_sb = [
            [const.tile([P, KH, BS], fp8, name=f"hT8_{p}_{n}", tag=f"hT8_{p}_{n}") for n in range(HN)]
            for p in range(2)
        ]
        hT16_sb = [
            [const.tile([P, KH, BS], bf16, name=f"hT16_{p}_{n}", tag=f"hT16_{p}_{n}") for n in range(HN)]
            for p in range(2)
        ]

        def hT16_slice(p, k):
            return hT16_sb[p][k // KH][:, k % KH, :]

        # Startup DMAs.  fp8 weights first (needed at t=0/t=1); bf16
        # copies are not needed until step T8, stream them last.
        for i in range(IT):
            nc.scalar.dma_start(wxh8_sb[:, i, :], wxh8[ts(i, P), :])
        nc.scalar.dma_start(biasrep_sb[:], biasrep[:, :])
        for k in range(KT):
            eng = nc.gpsimd if k % 2 == 0 else nc.scalar
            eng.dma_start(whh8_sb[:, k, :], whh8[ts(k, P), :])
        nc.gpsimd.dma_start(by_sb[:], b_y[:, :])
        nc.gpsimd.dma_start(ones_sb[:], ones[:, :])
        nc.gpsimd.dma_start(ident_sb[:], ident[:, :])
        for k in range(KT):
            nc.gpsimd.dma_start(why_sb[k][:], why[ts(k, P), :])
        # background: bf16 tail weights (not needed until step T8)
        for i in range(IT):
            nc.gpsimd.dma_start(wxh_sb[i][:], wxh[ts(i, P), :])
        for k in range(KT):
            nc.gpsimd.dma_start(whh_sb[k][:], whh[ts(k, P), :])

        def dma_x(t):
            """Allocate + start the x^T DMA for step t; returns the tile."""
            if t < T8:
                xt = xpool.tile([P, IT, BS], fp8, name="xt8", tag="xt8")
                nc.sync.dma_start(
                    xt[:], xT8[t, :, :].rearrange("(i p) b -> p i b", p=P)
                )
            else:
                xt = xpool.tile([P, IT, BS], bf16, name="xt16", tag="xt16")
                nc.sync.dma_start(
                    xt[:], xT16[t - T8, :, :].rearrange("(i p) b -> p i b", p=P)
                )
            return xt

        def emit_xproj(t, xt, ps, part=None):
            """Input-projection matmuls for step t into ps (start=True).
            part=0/1 emits only the first/second half of the contraction."""
            if t < T8:
                ips = range(IT // 2) if part is None else [part]
                for ip in ips:
                    for n in range(HN):
                        nc.tensor.matmul(
                            ps[n][:],
                            xt[:, 2 * ip : 2 * ip + 2, :],
                            wxh8_sb[:, 2 * ip : 2 * ip + 2, ts(n, NH)],
                            start=(ip == 0),
                            stop=(t == 0 and ip == IT // 2 - 1),
                            perf_mode=DR,
                        )
            else:
                iis = range(IT) if part is None else range(2 * part, 2 * part + 2)
                for i in iis:
                    for n in range(HN):
                        nc.tensor.matmul(
                            ps[n][:],
                            xt[:, i, :],
                            wxh_sb[i][:, ts(n, NH)],
                            start=(i == 0),
                            stop=(t == 0 and i == IT - 1),
                        )

        # --- software-pipelined recurrence ---
        # Iteration t PE order: [T(t) half0] [A(t+1)] [T(t) half1] [C(t)].
        xt_next = dma_x(0)
        if T_STEPS > 1:
            xt_next2 = dma_x(1)
        ps_next = [pspool.tile([P, NH], f32, name=f"ps{n}", tag="ps") for n in range(HN)]
        emit_xproj(0, xt_next, ps_next)

        prev_pre = None
        for t in range(T_STEPS):
            f8 = t < T8
            ps = ps_next
            # prefetch x for t+2 (2-step DMA lead)
            if t + 2 < T_STEPS:
                xt_next, xt_next2 = xt_next2, dma_x(t + 2)
            elif t + 1 < T_STEPS:
                xt_next = xt_next2

            if t > 0:
                hT_dst = hT8_sb if f8 else hT16_sb
                psT = [
                    psTpool.tile([P, KH * P], bf16, name=f"psT{n}", tag="psT")
                    for n in range(HN)
                ]
                # transposes, half 0
                for j in range(KH):
                    nc.tensor.transpose(
                        psT[0][:, ts(j, P)], prev_pre[:, ts(j, P)], ident_sb[:]
                    )
            # A(t+1): next step's input projection (independent PE work
            # that covers the evac->transpose->tanh chain of step t)
            if t + 1 < T_STEPS:
                ps_next = [
                    pspool.tile([P, NH], f32, name=f"ps{n}", tag="ps")
                    for n in range(HN)
                ]
                emit_xproj(t + 1, xt_next, ps_next)
            if t > 0:
                # transposes, half 1
                for j in range(KH):
                    nc.tensor.transpose(
                        psT[1][:, ts(j, P)],
                        prev_pre[:, ts(KH + j, P)],
                        ident_sb[:],
                    )
                # tanh: half0 per DoubleRow pair ([128,256] each, early
                # delivery for the first recurrence matmuls); half1 as one
                # [128,512] ACTIVATE (ends the tanh chain ~300ns sooner)
                for pp in range(2):
                    nc.scalar.activation(
                        hT_dst[t % 2][0][:, 2 * pp : 2 * pp + 2, :],
                        psT[0][:, ts(pp, 2 * P)].rearrange("p (k b) -> p k b", k=2),
                        TANH,
                    )
                nc.scalar.activation(
                    hT_dst[t % 2][1][:],
                    psT[1][:].rearrange("p (k b) -> p k b", k=KH),
                    TANH,
                )
                # C(t): recurrence.  Pair-major, n-interleaved, with n0
                # finishing two matmuls early for evac overlap.
                if f8:
                    order = [(0, 0), (0, 1), (1, 0), (2, 0), (3, 0), (1, 1), (2, 1), (3, 1)]
                    last = {0: (3, 0), 1: (3, 1)}
                    for kp, n in order:
                        buf = hT8_sb[t % 2][kp // 2]
                        nc.tensor.matmul(
                            ps[n][:],
                            buf[:, 2 * (kp % 2) : 2 * (kp % 2) + 2, :],
                            whh8_sb[:, 2 * kp : 2 * kp + 2, ts(n, NH)],
                            start=False,
                            stop=((kp, n) == last[n]),
                            perf_mode=DR,
                        )
                else:
                    order = [(k, n) for k in range(6) for n in range(HN)]
                    order += [(6, 0), (7, 0), (6, 1), (7, 1)]
                    last = {0: (7, 0), 1: (7, 1)}
                    for k, n in order:
                        nc.tensor.matmul(
                            ps[n][:],
                            hT16_slice(t % 2, k),
                            whh_sb[k][:, ts(n, NH)],
                            start=False,
                            stop=((k, n) == last[n]),
                        )
            # D(t): evacuate+bias on DVE per half (half0 first — its bank
            # completes two matmuls early and gates the next transposes)
            pre = hpool.tile([P, HID], bf16, name="pre", tag="pre")
            for n in range(HN):
                nc.vector.tensor_add(
                    pre[:, ts(n, NH)], ps[n][:], biasrep_sb[:, ts(n, NH)]
                )
            prev_pre = pre

        # --- final tanh + logits: out[b, n] = h_T[b, :] @ Why + b_y ---
        pb = T_STEPS % 2
        psTf = [
            psTpool.tile([P, KH * P], bf16, name=f"psTf{n}", tag="psT")
            for n in range(HN)
        ]
        for j in range(JT):
            nc.tensor.transpose(
                psTf[j // KH][:, ts(j % KH, P)], prev_pre[:, ts(j, P)], ident_sb[:]
            )
        for pp in range(JT // 2):
            half = pp // 2
            nc.scalar.activation(
                hT16_sb[pb][half][:, 2 * (pp % 2) : 2 * (pp % 2) + 2, :],
                psTf[half][:, ts(pp % 2, 2 * P)].rearrange("p (k b) -> p k b", k=2),
                TANH,
            )
        ps_l = pspool.tile([P, NCLS], f32, name="ps_l", tag="ps")
        for k in range(KT):
            nc.tensor.matmul(
                ps_l[:],
                hT16_slice(pb, k),
                why_sb[k][:],
                start=(k == 0),
                stop=False,
            )
        # broadcast-add b_y via a K=1 matmul: ones[1,P].T @ b_y[1,N]
        nc.tensor.matmul(ps_l[:], ones_sb[:], by_sb[:], start=False, stop=True)
        osb = opool.tile([P, NCLS], f32, name="osb", tag="osb")
        nc.vector.tensor_copy(osb[:], ps_l[:])
        nc.sync.dma_start(out[:, :], osb[:])


def _get_program():
    if "nc" not in _CACHE:
        nc, tensors = _build_nc()
        _emit(nc, tensors)
        nc.compile()
        _CACHE["nc"] = nc
    return _CACHE["nc"]


def _make_in_maps(x, Wxh, b_xh, Whh, b_hh, Why, b_y):
    import ml_dtypes

    bf = ml_dtypes.bfloat16
    f8 = ml_dtypes.float8_e4m3
    x = np.asarray(x, dtype=np.float32)
    Wxh_q = np.asarray(Wxh, np.float32).astype(bf)
    Whh_q = np.asarray(Whh, np.float32).astype(bf)
    Wxh8_q = np.asarray(Wxh, np.float32).astype(f8)
    Whh8_q = np.asarray(Whh, np.float32).astype(f8)
    Why_q = np.asarray(Why, np.float32).astype(bf)
    b_y_q = np.asarray(b_y, np.float32).reshape(1, NCLS).astype(bf)
    bias = np.asarray(b_xh, np.float32) + np.asarray(b_hh, np.float32)
    biasrep = np.ascontiguousarray(
        np.broadcast_to(bias.reshape(1, HID), (P, HID)), dtype=np.float32
    )
    ones_row = np.ones((1, P), dtype=bf)
    ident_np = np.eye(P, dtype=np.float32).astype(bf)

    # host-side layout marshaling: [B,T,IN] -> [T,IN,B], shard batch
    xT_full = np.transpose(x[:, :T_STEPS, :], (1, 2, 0))
    xT8_full = xT_full[:T8].astype(f8)
    xT16_full = xT_full[T8:].astype(bf)

    in_maps = []
    for c in range(NCORES):
        m = {
            "wxh": Wxh_q,
            "whh": Whh_q,
            "wxh8": Wxh8_q,
            "whh8": Whh8_q,
            "biasrep": biasrep,
            "why": Why_q,
            "b_y": b_y_q,
            "ones": ones_row,
            "ident": ident_np,
        }
        if T8 > 0:
            m["xT8"] = np.ascontiguousarray(xT8_full[:, :, c * BS : (c + 1) * BS])
        if T_STEPS > T8:
            m["xT16"] = np.ascontiguousarray(xT16_full[:, :, c * BS : (c + 1) * BS])
        in_maps.append(m)
    return in_maps


def kernel(x, Wxh, b_xh, Whh, b_hh, Why, b_y):
    from concourse.bass_utils import run_bass_kernel_spmd

    in_maps = _make_in_maps(x, Wxh, b_xh, Whh, b_hh, Why, b_y)
    nc = _get_program()
    res = run_bass_kernel_spmd(nc, in_maps, list(range(NCORES)))
    _CACHE["last_results"] = res
    return np.concatenate([res.results[c]["out"] for c in range(NCORES)], axis=0)


def bench_hw(inputs, iters=8):
    """Steady-state per-execution wall time with device-resident inputs."""
    import time as _time

    import jax
    from jax.sharding import Mesh, NamedSharding, PartitionSpec
    from jax.experimental.shard_map import shard_map
    from concourse import bass2jax, mybir

    in_maps = _make_in_maps(**inputs)
    nc = _get_program()
    bass2jax.install_neuronx_cc_hook()

    partition_name = (
        nc.partition_id_tensor.name if nc.partition_id_tensor else None
    )
    in_names, out_names, out_avals, zero_outs = [], [], [], []
    for alloc in nc.m.functions[0].allocations:
        if not isinstance(alloc, mybir.MemoryLocationSet):
            continue
        name = alloc.memorylocations[0].name
        if alloc.kind == "ExternalInput":
            if name != partition_name:
                in_names.append(name)
        elif alloc.kind == "ExternalOutput":
            out_names.append(name)
            np_dt = mybir.dt.np(alloc.dtype)
            out_avals.append(
                jax.core.ShapedArray(tuple(alloc.tensor_shape), np_dt)
            )
            zero_outs.append(np.zeros(tuple(alloc.tensor_shape), np_dt))
    n_params = len(in_names)
    all_in_names = in_names + out_names
    if partition_name is not None:
        all_in_names = all_in_names + [partition_name]

    def _body(*args):
        operands = list(args)
        if partition_name is not None:
            operands.append(bass2jax.partition_id_tensor())
        outs = bass2jax._bass_exec_p.bind(
            *operands,
            out_avals=tuple(out_avals),
            in_names=tuple(all_in_names),
            out_names=tuple(out_names),
            lowering_input_output_aliases=(),
            sim_require_finite=True,
            sim_require_nnan=True,
            nc=nc,
        )
        return tuple(outs)

    devices = jax.devices()[:NCORES]
    mesh = Mesh(np.asarray(devices), ("core",))
    spec = PartitionSpec("core")
    sharded = jax.jit(
        shard_map(
            _body,
            mesh=mesh,
            in_specs=(spec,) * (n_params + len(out_names)),
            out_specs=(spec,) * len(out_names),
            check_rep=False,
        ),
        keep_unused=True,
    )
    concat_in = [
        jax.device_put(
            np.concatenate([np.asarray(m[nm]) for m in in_maps], axis=0),
            NamedSharding(mesh, spec),
        )
        for nm in in_names
    ]
    concat_zero = [
        jax.device_put(
            np.zeros((NCORES * z.shape[0], *z.shape[1:]), z.dtype),
            NamedSharding(mesh, spec),
        )
        for z in zero_outs
    ]
    jax.block_until_ready(concat_in)

    # warmup (compile)
    outs = sharded(*concat_in, *concat_zero)
    jax.block_until_ready(outs)

    times = []
    for _ in range(iters):
        t0 = _time.perf_counter()
        outs = sharded(*concat_in, *concat_zero)
        jax.block_until_ready(outs)
        times.append((_time.perf_counter() - t0) * 1e9)
    result = np.concatenate(
        [
            np.asarray(outs[0]).reshape(NCORES, BS, NCLS)[c]
            for c in range(NCORES)
        ],
        axis=0,
    )
    return min(times), times, result
